# revision 5
# baseline (speedup 1.0000x reference)
"""Trainium2 Bass kernel for nn_Cholesky_from_z (pair-compressed log/exp).

Closed form: L[i,j] = z[i,j] * sqrt(prod_{k<j}(1-z[i,k]^2)) (j<i),
L[i,i] = sqrt(prod_{k<i}(...)) -- exclusive cumprod of a=(1-z^2) along each
matrix row. Computed in LOG space with the serial scan replaced by a matmul
on the (otherwise idle) tensor engine, and positions PAIR-COMPRESSED so the
activation engine (ln/exp) and PE see half the elements:

  per pair m (slots 2m,2m+1):  pp[m] = a[2m]*a[2m+1]
  C[m] = 0.5 * sum_{m'<m, same row} ln pp[m']      (matmul vs const S)
  L_even = z_even * exp(C)
  L_odd  = z_odd  * exp(C) * sqrt(a_even)

Layout: TRANSPOSED -- partition = pair position, free = sample. Matrix rows
are padded to even length and bin-packed into 65 blocks of 128 slots
(= 33 pair-blocks of 128 pairs; pairs never cross rows), so the segmented
exclusive cumsum is one 128x128 matmul per pair-block, no carries.

Per pair-block (tiles [128, 256] fp16, PSUM fp32):
  ue=ze*ze; uo=zo*zo; aen=ue-1                DVE
  pp=(uo-1)*(ue-1); sqe=(-1*aen)^0.5          DVE (scalar_tensor_tensor/pow)
  w = Ln(pp)                                  ACT   } both in act table 6 =>
  ge = Exp(S_b^T @ w)                         PE+ACT} single table load
  le=ze*ge; go=sqe*ge; lo=zo*go               DVE
Diagonal sentinel z=0.998 (<1 keeps Ln finite); host divides diag by it.
Batch 2048 sharded 256 samples/core over 8 cores; fp16 I/O (tol 2e-2,
measured ~4.4e-4).
"""

import sys

if "/opt/trn_rl_repo" not in sys.path:
    sys.path.insert(0, "/opt/trn_rl_repo")

import numpy as np

B = 2048
N = 128
NZ = N * (N - 1) // 2          # 8128
NBLK = 65
PACK = NBLK * 128              # 8320 (even-aligned rows)
NPAIR = PACK // 2              # 4160
PBLK = 33
PPAD = PBLK * 128              # 4224
NCORES = 8
SAMP = B // NCORES             # 256
SENT = np.float16(0.998)
GROUPS = [(0, 4), (4, 12), (12, 22), (22, PBLK)]   # pair-block groups
BATCH = 4                                           # psum/exp granularity

# --- host-side packing maps -------------------------------------------------
def _build_maps():
    row_of_block = [(i, 125 - i) for i in range(63)] + [(126,), (127,)]
    slot_row = np.full(PACK, -1, np.int64)
    slot_col = np.full(PACK, -1, np.int64)
    for b, rows in enumerate(row_of_block):
        pos = b * 128
        for r in rows:
            L = r + 1
            slot_row[pos:pos + L] = r
            slot_col[pos:pos + L - 1] = np.arange(r)
            slot_col[pos + L - 1] = r              # diag/sentinel slot
            pos += L + (1 if r % 2 == 0 else 0)    # pad rows to even length
    return slot_row, slot_col

_slot_row, _slot_col = _build_maps()
_valid = _slot_row >= 0
_strict = _valid & (_slot_col < _slot_row)
_diag = _valid & (_slot_col == _slot_row)
_tri_idx = (_slot_row[_strict] * (_slot_row[_strict] - 1) // 2
            + _slot_col[_strict])

def _build_S():
    pr = np.full(PPAD, -1, np.int64)
    pr[:NPAIR] = _slot_row[0::2]
    S = np.zeros((PBLK, 128, 128), np.float16)
    k = np.arange(128)
    for q in range(PBLK):
        s = pr[q * 128:(q + 1) * 128]
        same = (s[:, None] == s[None, :]) & (s[:, None] >= 0)
        S[q] = np.where(same & (k[:, None] < k[None, :]),
                        np.float16(0.5), np.float16(0))
    return np.ascontiguousarray(S.transpose(1, 0, 2))   # [k, blk, t]

_S_host = _build_S()

_prog_cache = {}


def _build_program():
    import concourse.bacc as bacc
    import concourse.mybir as mybir
    import bass_rust
    from concourse.tile import TileContext

    f16 = mybir.dt.float16
    f32 = mybir.dt.float32
    Act = mybir.ActivationFunctionType
    Alu = mybir.AluOpType

    nc = bacc.Bacc("TRN2", target_bir_lowering=False, debug=False,
                   num_devices=NCORES)
    ze_d = nc.dram_tensor("ze", [128, PBLK, SAMP], f16,
                          kind="ExternalInput").ap()
    zo_d = nc.dram_tensor("zo", [128, PBLK, SAMP], f16,
                          kind="ExternalInput").ap()
    sc_d = nc.dram_tensor("sc", [128, PBLK, 128], f16,
                          kind="ExternalInput").ap()
    lp_d = nc.dram_tensor("lp", [128, PBLK, 2, SAMP], f16,
                          kind="ExternalOutput").ap()

    def load_table(set_id):
        _tl = bass_rust.InstLoadActFuncSet(
            name=nc.get_next_instruction_name(), ins=[], outs=[],
            act_func_set_id=set_id)
        nc.scalar.add_instruction(_tl)

    with TileContext(nc) as tc:
        with (
            tc.tile_pool(name="sb", bufs=1) as sb,
            tc.psum_pool(name="ps", bufs=4) as pp_pool,
        ):
            ze = sb.tile([128, PBLK, SAMP], f16)
            zo = sb.tile([128, PBLK, SAMP], f16)
            ue = sb.tile([128, PBLK, SAMP], f16)
            uo = sb.tile([128, PBLK, SAMP], f16)
            aen = sb.tile([128, PBLK, SAMP], f16)
            ppt = sb.tile([128, PBLK, SAMP], f16)
            wt = sb.tile([128, PBLK, SAMP], f16)
            sqe = sb.tile([128, PBLK, SAMP], f16)
            ge = sb.tile([128, PBLK, SAMP], f16)
            lt = sb.tile([128, PBLK, 2, SAMP], f16)
            st = sb.tile([128, PBLK, 128], f16)

            # Phase A: sqrt table; all sqe = Sqrt(1-ze^2) (needs only ze).
            # ze DMAs are issued first so this phase drains early.
            load_table(3)                      # sqrt_and_others
            for a0, a1 in [(0, 16), (16, PBLK)]:
                a = (slice(None), slice(a0, a1), slice(None))
                nc.sync.dma_start(out=ze[a], in_=ze_d[a])
                nc.vector.tensor_mul(ue[a], ze[a], ze[a])
                nc.scalar.activation(sqe[a], ue[a], Act.Sqrt,
                                     bias=1.0, scale=-1.0)
            # Phase B: ln+exp table (set 6) for everything else.
            load_table(6)
            done = 0
            for g0, g1 in GROUPS:
                g = (slice(None), slice(g0, g1), slice(None))
                nc.sync.dma_start(out=zo[g], in_=zo_d[g])
                nc.sync.dma_start(out=st[g], in_=sc_d[g])
                nc.vector.tensor_mul(uo[g], zo[g], zo[g])
                nc.vector.tensor_scalar_sub(aen[g], ue[g], 1.0)
                # pp = (uo-1)*(ue-1) = (1-ze^2)(1-zo^2)
                nc.vector.scalar_tensor_tensor(
                    ppt[g], uo[g], 1.0, aen[g], Alu.subtract, Alu.mult)
                nc.scalar.activation(wt[g], ppt[g], Act.Ln)
                while done < g1 and (g1 - done >= BATCH or g1 == PBLK):
                    b0, b1 = done, min(done + BATCH, g1)
                    nb = b1 - b0
                    pt = pp_pool.tile([128, BATCH, SAMP], f32, tag="ps")
                    for j in range(nb):
                        nc.tensor.matmul(pt[:, j, :], st[:, b0 + j, :],
                                         wt[:, b0 + j, :])
                    bsl = (slice(None), slice(b0, b1), slice(None))
                    nc.scalar.activation(ge[bsl], pt[:, 0:nb, :], Act.Exp)
                    nc.vector.tensor_mul(lt[:, b0:b1, 0, :], ze[bsl], ge[bsl])
                    # go reuses aen's slots (aen dead after pp/sqe)
                    nc.vector.tensor_mul(aen[bsl], sqe[bsl], ge[bsl])
                    nc.vector.tensor_mul(lt[:, b0:b1, 1, :], zo[bsl], aen[bsl])
                    nc.sync.dma_start(out=lp_d[:, b0:b1, :, :],
                                      in_=lt[:, b0:b1, :, :])
                    done = b1
    nc.compile()
    return nc


def _get_program():
    if "nc" not in _prog_cache:
        _prog_cache["nc"] = _build_program()
    return _prog_cache["nc"]


def _to_core(a):
    # [SAMP, PPAD] -> [128, PBLK, SAMP]
    return np.ascontiguousarray(
        a.T.reshape(PBLK, 128, SAMP).transpose(1, 0, 2))


def kernel(inputs: np.ndarray, _return_raw=False, **run_kw) -> np.ndarray:
    from concourse.bass_utils import run_bass_kernel_spmd

    assert inputs.shape == (B, NZ), inputs.shape
    zvec = inputs.astype(np.float16)

    zpk = np.zeros((B, PACK), np.float16)
    zpk[:, _strict] = zvec[:, _tri_idx]
    zpk[:, _diag] = SENT
    ze_all = np.zeros((B, PPAD), np.float16)
    zo_all = np.zeros((B, PPAD), np.float16)
    ze_all[:, :NPAIR] = zpk[:, 0::2]
    zo_all[:, :NPAIR] = zpk[:, 1::2]

    in_maps = []
    for c in range(NCORES):
        sl = slice(c * SAMP, (c + 1) * SAMP)
        in_maps.append({"ze": _to_core(ze_all[sl]),
                        "zo": _to_core(zo_all[sl]),
                        "sc": _S_host})

    nc = _get_program()
    res = run_bass_kernel_spmd(nc, in_maps, list(range(NCORES)), **run_kw)

    lpk = np.zeros((B, PACK), np.float32)
    for c in range(NCORES):
        lc = res.results[c]["lp"]               # [128, PBLK, 2, SAMP]
        le = lc[:, :, 0, :].transpose(1, 0, 2).reshape(PPAD, SAMP).T
        lo = lc[:, :, 1, :].transpose(1, 0, 2).reshape(PPAD, SAMP).T
        sl = slice(c * SAMP, (c + 1) * SAMP)
        lpk[sl, 0::2] = le[:, :NPAIR]
        lpk[sl, 1::2] = lo[:, :NPAIR]

    out = np.zeros((B, N, N), np.float32)
    out[:, _slot_row[_strict], _slot_col[_strict]] = lpk[:, _strict]
    out[:, _slot_row[_diag], _slot_col[_diag]] = (
        lpk[:, _diag] / np.float32(SENT))
    if _return_raw:
        return out, res
    return out


# revision 6
# speedup vs baseline: 1.1056x; 1.1056x over previous
"""Trainium2 Bass kernel for nn_Cholesky_from_z (pair-compressed log/exp).

Closed form: L[i,j] = z[i,j] * sqrt(prod_{k<j}(1-z[i,k]^2)) (j<i),
L[i,i] = sqrt(prod_{k<i}(...)) -- exclusive cumprod of a=(1-z^2) along each
matrix row. Computed in LOG space with the serial scan replaced by a matmul
on the (otherwise idle) tensor engine, and positions PAIR-COMPRESSED so the
activation engine (ln/exp) and PE see half the elements:

  per pair m (slots 2m,2m+1):  pp[m] = a[2m]*a[2m+1]
  C[m] = 0.5 * sum_{m'<m, same row} ln pp[m']      (matmul vs const S)
  L_even = z_even * exp(C)
  L_odd  = z_odd  * exp(C) * sqrt(a_even)

Layout: TRANSPOSED -- partition = pair position, free = sample. Matrix rows
are padded to even length and bin-packed into 65 blocks of 128 slots
(= 33 pair-blocks of 128 pairs; pairs never cross rows), so the segmented
exclusive cumsum is one 128x128 matmul per pair-block, no carries.

Per pair-block (tiles [128, 256] fp16, PSUM fp32):
  ue=ze*ze; uo=zo*zo; aen=ue-1                DVE
  pp=(uo-1)*(ue-1); sqe=(-1*aen)^0.5          DVE (scalar_tensor_tensor/pow)
  w = Ln(pp)                                  ACT   } both in act table 6 =>
  ge = Exp(S_b^T @ w)                         PE+ACT} single table load
  le=ze*ge; go=sqe*ge; lo=zo*go               DVE
Diagonal sentinel z=0.998 (<1 keeps Ln finite); host divides diag by it.
Batch 2048 sharded 256 samples/core over 8 cores; fp16 I/O (tol 2e-2,
measured ~4.4e-4).
"""

import sys

if "/opt/trn_rl_repo" not in sys.path:
    sys.path.insert(0, "/opt/trn_rl_repo")

import numpy as np

B = 2048
N = 128
NZ = N * (N - 1) // 2          # 8128
NBLK = 65
PACK = NBLK * 128              # 8320 (even-aligned rows)
NPAIR = PACK // 2              # 4160
PBLK = 33
PPAD = PBLK * 128              # 4224
NCORES = 8
SAMP = B // NCORES             # 256
SENT = np.float16(0.998)
GROUPS = [(0, 4), (4, 12), (12, 22), (22, PBLK)]   # pair-block groups
BATCH = 4                                           # psum/exp granularity

# --- host-side packing maps -------------------------------------------------
def _build_maps():
    row_of_block = [(i, 125 - i) for i in range(63)] + [(126,), (127,)]
    slot_row = np.full(PACK, -1, np.int64)
    slot_col = np.full(PACK, -1, np.int64)
    for b, rows in enumerate(row_of_block):
        pos = b * 128
        for r in rows:
            L = r + 1
            slot_row[pos:pos + L] = r
            slot_col[pos:pos + L - 1] = np.arange(r)
            slot_col[pos + L - 1] = r              # diag/sentinel slot
            pos += L + (1 if r % 2 == 0 else 0)    # pad rows to even length
    return slot_row, slot_col

_slot_row, _slot_col = _build_maps()
_valid = _slot_row >= 0
_strict = _valid & (_slot_col < _slot_row)
_diag = _valid & (_slot_col == _slot_row)
_tri_idx = (_slot_row[_strict] * (_slot_row[_strict] - 1) // 2
            + _slot_col[_strict])

def _build_S():
    pr = np.full(PPAD, -1, np.int64)
    pr[:NPAIR] = _slot_row[0::2]
    S = np.zeros((PBLK, 128, 128), np.float16)
    k = np.arange(128)
    for q in range(PBLK):
        s = pr[q * 128:(q + 1) * 128]
        same = (s[:, None] == s[None, :]) & (s[:, None] >= 0)
        S[q] = np.where(same & (k[:, None] < k[None, :]),
                        np.float16(0.5), np.float16(0))
    return np.ascontiguousarray(S.transpose(1, 0, 2))   # [k, blk, t]

_S_host = _build_S()

_prog_cache = {}


def _build_program():
    import concourse.bacc as bacc
    import concourse.mybir as mybir
    import bass_rust
    from concourse.tile import TileContext

    f16 = mybir.dt.float16
    f32 = mybir.dt.float32
    Act = mybir.ActivationFunctionType
    Alu = mybir.AluOpType

    nc = bacc.Bacc("TRN2", target_bir_lowering=False, debug=False,
                   num_devices=NCORES)
    ze_d = nc.dram_tensor("ze", [128, PBLK, SAMP], f16,
                          kind="ExternalInput").ap()
    zo_d = nc.dram_tensor("zo", [128, PBLK, SAMP], f16,
                          kind="ExternalInput").ap()
    sc_d = nc.dram_tensor("sc", [128, PBLK, 128], f16,
                          kind="ExternalInput").ap()
    lp_d = nc.dram_tensor("lp", [128, PBLK, 2, SAMP], f16,
                          kind="ExternalOutput").ap()

    def load_table(set_id):
        _tl = bass_rust.InstLoadActFuncSet(
            name=nc.get_next_instruction_name(), ins=[], outs=[],
            act_func_set_id=set_id)
        nc.scalar.add_instruction(_tl)

    with TileContext(nc) as tc:
        with (
            tc.tile_pool(name="sb", bufs=1) as sb,
            tc.psum_pool(name="ps", bufs=4) as pp_pool,
        ):
            # single act table (set 6: ln+exp+square+copy) serves every
            # activation below -- no table swaps.
            load_table(6)

            ze = sb.tile([128, PBLK, SAMP], f16)
            zo = sb.tile([128, PBLK, SAMP], f16)
            ue = sb.tile([128, PBLK, SAMP], f16)
            uo = sb.tile([128, PBLK, SAMP], f16)
            w2 = sb.tile([128, PBLK, SAMP], f16)
            w3 = sb.tile([128, PBLK, SAMP], f16)
            wt = sb.tile([128, PBLK, SAMP], f16)
            sqe = sb.tile([128, PBLK, SAMP], f16)
            ge = sb.tile([128, PBLK, SAMP], f16)
            lt = sb.tile([128, PBLK, 2, SAMP], f16)
            st = sb.tile([128, PBLK, 128], f16)

            done = 0
            for g0, g1 in GROUPS:
                g = (slice(None), slice(g0, g1), slice(None))
                nc.sync.dma_start(out=ze[g], in_=ze_d[g])
                nc.sync.dma_start(out=zo[g], in_=zo_d[g])
                nc.sync.dma_start(out=st[g], in_=sc_d[g])
                nc.vector.tensor_mul(ue[g], ze[g], ze[g])
                nc.vector.tensor_mul(uo[g], zo[g], zo[g])
                nc.scalar.activation(w2[g], ue[g], Act.Ln,
                                     bias=1.0, scale=-1.0)   # ln(1-ze^2)
                nc.scalar.activation(w3[g], uo[g], Act.Ln,
                                     bias=1.0, scale=-1.0)   # ln(1-zo^2)
                nc.vector.tensor_add(wt[g], w2[g], w3[g])    # ln(pp)
                nc.scalar.activation(sqe[g], w2[g], Act.Exp,
                                     scale=0.5)              # sqrt(1-ze^2)
                while done < g1 and (g1 - done >= BATCH or g1 == PBLK):
                    b0, b1 = done, min(done + BATCH, g1)
                    nb = b1 - b0
                    pt = pp_pool.tile([128, BATCH, SAMP], f32, tag="ps")
                    for j in range(nb):
                        nc.tensor.matmul(pt[:, j, :], st[:, b0 + j, :],
                                         wt[:, b0 + j, :])
                    bsl = (slice(None), slice(b0, b1), slice(None))
                    nc.scalar.activation(ge[bsl], pt[:, 0:nb, :], Act.Exp)
                    nc.vector.tensor_mul(lt[:, b0:b1, 0, :], ze[bsl], ge[bsl])
                    # go reuses w3's slots (w3 dead after wt)
                    nc.vector.tensor_mul(w3[bsl], sqe[bsl], ge[bsl])
                    nc.vector.tensor_mul(lt[:, b0:b1, 1, :], zo[bsl], w3[bsl])
                    nc.sync.dma_start(out=lp_d[:, b0:b1, :, :],
                                      in_=lt[:, b0:b1, :, :])
                    done = b1
    nc.compile()
    return nc


def _get_program():
    if "nc" not in _prog_cache:
        _prog_cache["nc"] = _build_program()
    return _prog_cache["nc"]


def _to_core(a):
    # [SAMP, PPAD] -> [128, PBLK, SAMP]
    return np.ascontiguousarray(
        a.T.reshape(PBLK, 128, SAMP).transpose(1, 0, 2))


def kernel(inputs: np.ndarray, _return_raw=False, **run_kw) -> np.ndarray:
    from concourse.bass_utils import run_bass_kernel_spmd

    assert inputs.shape == (B, NZ), inputs.shape
    zvec = inputs.astype(np.float16)

    zpk = np.zeros((B, PACK), np.float16)
    zpk[:, _strict] = zvec[:, _tri_idx]
    zpk[:, _diag] = SENT
    ze_all = np.zeros((B, PPAD), np.float16)
    zo_all = np.zeros((B, PPAD), np.float16)
    ze_all[:, :NPAIR] = zpk[:, 0::2]
    zo_all[:, :NPAIR] = zpk[:, 1::2]

    in_maps = []
    for c in range(NCORES):
        sl = slice(c * SAMP, (c + 1) * SAMP)
        in_maps.append({"ze": _to_core(ze_all[sl]),
                        "zo": _to_core(zo_all[sl]),
                        "sc": _S_host})

    nc = _get_program()
    res = run_bass_kernel_spmd(nc, in_maps, list(range(NCORES)), **run_kw)

    lpk = np.zeros((B, PACK), np.float32)
    for c in range(NCORES):
        lc = res.results[c]["lp"]               # [128, PBLK, 2, SAMP]
        le = lc[:, :, 0, :].transpose(1, 0, 2).reshape(PPAD, SAMP).T
        lo = lc[:, :, 1, :].transpose(1, 0, 2).reshape(PPAD, SAMP).T
        sl = slice(c * SAMP, (c + 1) * SAMP)
        lpk[sl, 0::2] = le[:, :NPAIR]
        lpk[sl, 1::2] = lo[:, :NPAIR]

    out = np.zeros((B, N, N), np.float32)
    out[:, _slot_row[_strict], _slot_col[_strict]] = lpk[:, _strict]
    out[:, _slot_row[_diag], _slot_col[_diag]] = (
        lpk[:, _diag] / np.float32(SENT))
    if _return_raw:
        return out, res
    return out
